# revision 15
# baseline (speedup 1.0000x reference)
"""TransformerXL relative attention on 8 TRN2 NeuronCores, data-parallel over batch.

Problem shapes (hardcoded): B=8, Q=512, M=512, R=1024, HIDDEN=1024, HEADS=16, SPH=64.
Each core computes one batch element end to end; no collectives.

Layout strategy: host passes transposed activations (refT/queryT/posT, [D, *]) so
every matmul has its contraction dim on partitions. rel_shift is exact via a padded
DRAM buffer: writing positions rows into [Q, R+1] (pad col 0) makes the shifted
tensor a contiguous read at element offset Q. The token mask is folded into the
padded buffer on the host (inverse-shifted), so masking costs nothing on device.
Softmax runs without max-subtraction (scores are O(+-30), exp is safe in f32).
"""
import numpy as np
import ml_dtypes

HIDDEN = 1024
HEADS = 16
SPH = 64
B, Q, M = 8, 512, 512
R = Q + M
NEG_INF = -1e9
P = 128
NPAIR = 8   # head pairs
NQT = Q // P
NCH = HIDDEN // P

_CACHE = {}


def _build_nc(n_iter=1):
    import concourse.bass as bass  # noqa: F401
    from concourse import bacc
    import concourse.tile as tile
    import concourse.mybir as mybir

    f32 = mybir.dt.float32
    f32r = mybir.dt.float32r
    bf16 = mybir.dt.bfloat16
    EXP = mybir.ActivationFunctionType.Exp
    IDENT = mybir.ActivationFunctionType.Identity

    nc = bacc.Bacc("TRN2", target_bir_lowering=False, debug=False)

    refT_e = nc.declare_dram_parameter("refT", [HIDDEN, R], bf16, isOutput=False)
    queryT_e = nc.declare_dram_parameter("queryT", [HIDDEN, Q], bf16, isOutput=False)
    posT_e = nc.declare_dram_parameter("posT", [HIDDEN, R], bf16, isOutput=False)
    wq_e = nc.declare_dram_parameter("wq", [HIDDEN, HIDDEN], bf16, isOutput=False)
    wkc_e = nc.declare_dram_parameter("wkc", [HIDDEN, HIDDEN], bf16, isOutput=False)
    wkp_e = nc.declare_dram_parameter("wkp", [HIDDEN, HIDDEN], bf16, isOutput=False)
    wv_e = nc.declare_dram_parameter("wv", [HIDDEN, HIDDEN], bf16, isOutput=False)
    wo_e = nc.declare_dram_parameter("wo", [HIDDEN, HIDDEN], bf16, isOutput=False)
    cbp_e = nc.declare_dram_parameter("cbp", [P, NPAIR], f32, isOutput=False)
    pbp_e = nc.declare_dram_parameter("pbp", [P, NPAIR], f32, isOutput=False)
    mshift_e = nc.declare_dram_parameter("mshift", [Q, R], bf16, isOutput=False)
    mcol_e = nc.declare_dram_parameter("mcol", [Q, 1], bf16, isOutput=False)
    out_e = nc.declare_dram_parameter("out", [Q, HIDDEN], f32, isOutput=True)

    with tile.TileContext(nc) as tc:
        from contextlib import ExitStack
        ctx = ExitStack()
        dram = ctx.enter_context(tc.tile_pool(name="dram", bufs=1, space="DRAM"))
        # per-head padded DRAM buffers for the rel_shift round trip (tile pool so
        # Tile tracks write->read deps, incl. the tile-crossing wrap reads)
        pads = [dram.tile([Q * (R + 1)], bf16, tag=f"pad{h}", name=f"pad{h}")
                for h in range(HEADS)]
        pad_rows = [t[:].rearrange("(q c) -> q c", c=R + 1) for t in pads]
        shift_views = [t[Q:Q + Q * R].rearrange("(q c) -> q c", c=R) for t in pads]
        const = ctx.enter_context(tc.tile_pool(name="const", bufs=1))
        resid = ctx.enter_context(tc.tile_pool(name="resid", bufs=1))
        wstream = ctx.enter_context(tc.tile_pool(name="wstream", bufs=2))
        psum = ctx.enter_context(tc.tile_pool(name="psum", bufs=1, space="PSUM"))
        work = ctx.enter_context(tc.tile_pool(name="work", bufs=2))
        small = ctx.enter_context(tc.tile_pool(name="small", bufs=3))

        # ---- pad-column prefill (one time, tiny) ----
        with nc.allow_non_contiguous_dma(reason="one-time pad columns"):
            for h in range(HEADS):
                nc.sync.dma_start(pad_rows[h][:, 0:1], mcol_e[:, :])

        # ---- resident loads ----
        cbp = const.tile([P, NPAIR], f32, tag="cbp")
        nc.sync.dma_start(cbp[:], cbp_e[:, :])
        pbp = const.tile([P, NPAIR], f32, tag="pbp")
        nc.sync.dma_start(pbp[:], pbp_e[:, :])
        mshift_sb = []
        for qt in range(NQT):
            mt = const.tile([P, R], bf16, tag=f"mshift{qt}")
            nc.sync.dma_start(mt[:], mshift_e[qt * P:(qt + 1) * P, :])
            mshift_sb.append(mt)
        refT_sb = []
        for c in range(NCH):
            t = resid.tile([P, R], bf16, tag=f"refT{c}")
            nc.sync.dma_start(t[:], refT_e[c * P:(c + 1) * P, :])
            refT_sb.append(t)
        posT_sb = []
        for c in range(NCH):
            t = resid.tile([P, R], bf16, tag=f"posT{c}")
            nc.sync.dma_start(t[:], posT_e[c * P:(c + 1) * P, :])
            posT_sb.append(t)
        queryT_sb = []
        for c in range(NCH):
            t = resid.tile([P, Q], bf16, tag=f"queryT{c}")
            nc.sync.dma_start(t[:], queryT_e[c * P:(c + 1) * P, :])
            queryT_sb.append(t)

        import numpy as _np
        import ml_dtypes as _mld
        ident_d = nc.inline_tensor(_np.eye(P, dtype=_mld.bfloat16), name="ident_d")
        ident = const.tile([P, P], bf16, tag="ident", name="ident")
        nc.sync.dma_start(ident[:], ident_d[:, :])

        for _it in range(n_iter):
            _build_body(nc, tc, mybir, ctx, const, resid, wstream, psum, work,
                        small, dram, pads, pad_rows, shift_views, cbp, pbp,
                        mshift_sb, refT_sb, posT_sb, queryT_sb,
                        wq_e, wkc_e, wkp_e, wv_e, wo_e, out_e, ident)
        ctx.close()

    nc.compile()
    return nc


def _build_body(nc, tc, mybir, ctx, const, resid, wstream, psum, work, small,
                dram, pads, pad_rows, shift_views, cbp, pbp, mshift_sb,
                refT_sb, posT_sb, queryT_sb, wq_e, wkc_e, wkp_e, wv_e, wo_e,
                out_e, ident):
        f32 = mybir.dt.float32
        bf16 = mybir.dt.bfloat16
        EXP = mybir.ActivationFunctionType.Exp
        IDENT = mybir.ActivationFunctionType.Identity
        VW = 65  # 64 v columns + 1 ones column per head (softmax denominator)

        # ---- stage V: v_sb[rt][:, 65h:65h+64] = (ref @ Wv)[rt], col 65h+64 = 1
        v_sb = []
        for rt in range(NCH):
            t = resid.tile([P, HEADS * VW], bf16, tag=f"v{rt}", name=f"v{rt}")
            nc.vector.memset(t[:].rearrange("p (h w) -> p h w", w=VW)[:, :, 64:65],
                             1.0)
            v_sb.append(t)
        wvts = []
        for c in range(NCH):
            t = wstream.tile([P, HIDDEN], bf16, tag=f"wvh{c}", bufs=1)
            nc.sync.dma_start(t[:], wv_e[c * P:(c + 1) * P, :])
            wvts.append(t)
        for rt in range(NCH):
            for half in range(2):
                vps = psum.tile([P, 512], f32, tag="mm512", bufs=2)
                for c in range(NCH):
                    nc.tensor.matmul(vps[:], refT_sb[c][:, rt * P:(rt + 1) * P],
                                     wvts[c][:, half * 512:(half + 1) * 512],
                                     start=(c == 0), stop=(c == NCH - 1))
                # strided copy into the 65-wide head slots
                dst = v_sb[rt][:, half * 8 * VW:(half * 8 + 8) * VW]
                dst = dst.rearrange("p (h w) -> p h w", w=VW)[:, :, 0:64]
                nc.vector.tensor_copy(dst, vps[:].rearrange("p (h w) -> p h w", w=64))

        ones_row = const.tile([1, 64], f32, tag="ones_row", name="ones_row")
        nc.vector.memset(ones_row[:], 1.0)

        # ---- per head-pair ----
        oT_sb = []
        for p in range(NPAIR):
            oT_sb.append(resid.tile([P, Q], bf16, tag=f"oT{p}", name=f"oT{p}"))

        for p in range(NPAIR):
            hs0 = p * P
            # pair-column weight loads: one DMA each, [128, 8*128] with chunk c
            # at columns [c*128, (c+1)*128)
            def _pair_w(w_e, name):
                t = wstream.tile([P, HIDDEN], bf16, tag=f"wp_{name}", bufs=2,
                                 name=f"wp_{name}")
                src = w_e[:, hs0:hs0 + P].rearrange("(c p) m -> p c m", p=P)
                nc.sync.dma_start(t[:].rearrange("p (c m) -> p c m", m=P), src)
                return t
            wkct = _pair_w(wkc_e, "kc")
            wkpt = _pair_w(wkp_e, "kp")
            wqt = _pair_w(wq_e, "q")

            kc_sb = work.tile([P, R], bf16, tag="kc_sb")
            kp_sb = work.tile([P, R], bf16, tag="kp_sb")
            for rhalf in range(2):
                ps = psum.tile([P, 512], f32, tag="mm512", bufs=2)
                for c in range(NCH):
                    nc.tensor.matmul(ps[:], wkct[:, c * P:(c + 1) * P],
                                     refT_sb[c][:, rhalf * 512:(rhalf + 1) * 512],
                                     start=(c == 0), stop=(c == NCH - 1))
                nc.vector.tensor_copy(kc_sb[:, rhalf * 512:(rhalf + 1) * 512], ps[:])
            for rhalf in range(2):
                ps = psum.tile([P, 512], f32, tag="mm512", bufs=2)
                for c in range(NCH):
                    nc.tensor.matmul(ps[:], wkpt[:, c * P:(c + 1) * P],
                                     posT_sb[c][:, rhalf * 512:(rhalf + 1) * 512],
                                     start=(c == 0), stop=(c == NCH - 1))
                nc.scalar.activation(kp_sb[:, rhalf * 512:(rhalf + 1) * 512], ps[:],
                                     IDENT, bias=0.0, scale=1.0)
            qc_sb = work.tile([P, Q], bf16, tag="qc_sb")
            qp_sb = work.tile([P, Q], bf16, tag="qp_sb")
            ps = psum.tile([P, 512], f32, tag="mm512", bufs=2)
            for c in range(NCH):
                nc.tensor.matmul(ps[:], wqt[:, c * P:(c + 1) * P], queryT_sb[c][:],
                                 start=(c == 0), stop=(c == NCH - 1))
            nc.scalar.activation(qc_sb[:], ps[:], IDENT, bias=cbp[:, p:p + 1],
                                 scale=1.0)
            nc.scalar.activation(qp_sb[:], ps[:], IDENT, bias=pbp[:, p:p + 1],
                                 scale=1.0)

            for u in range(2):
                h = 2 * p + u
                s0 = u * 64
                # pass 1: position scores [q, r] -> padded DRAM rows (bf16)
                for qt in range(NQT):
                    pps = psum.tile([P, R], f32, tag="pps", bufs=1)
                    for rhalf in range(2):
                        nc.tensor.matmul(
                            pps[:, rhalf * 512:(rhalf + 1) * 512],
                            qp_sb[s0:s0 + 64, qt * P:(qt + 1) * P],
                            kp_sb[s0:s0 + 64, rhalf * 512:(rhalf + 1) * 512],
                            start=True, stop=True)
                    padt = work.tile([P, R], bf16, tag="padt")
                    nc.vector.tensor_add(padt[:], pps[:], mshift_sb[qt][:])
                    nc.scalar.dma_start(pad_rows[h][qt * P:(qt + 1) * P, 1:], padt[:])
                # pass 2: one wide transposed shifted read, PE identity-add
                # of shifted into content PSUM, exp from PSUM, PV
                shA = work.tile([P, NCH * 512], bf16, tag="shA", bufs=2)
                nc.scalar.dma_start(
                    shA[:].rearrange("p (b q) -> p b q", q=512),
                    shift_views[h][:, :], transpose=True)
                opsT = psum.tile([VW, 512], f32, tag="opsT", bufs=1)
                for rb in range(NCH):
                    cpsT = psum.tile([P, 512], f32, tag="cpsT", bufs=2)
                    nc.tensor.matmul(cpsT[:],
                                     kc_sb[s0:s0 + 64, rb * P:(rb + 1) * P],
                                     qc_sb[s0:s0 + 64, :], start=True, stop=False,
                                     skip_group_check=True)
                    nc.tensor.matmul(cpsT[:], ident[:],
                                     shA[:, rb * 512:(rb + 1) * 512],
                                     start=False, stop=True, skip_group_check=True)
                    eT = work.tile([P, 512], bf16, tag="eT", bufs=3)
                    nc.scalar.activation(eT[:], cpsT[:], EXP, bias=0.0, scale=1.0)
                    nc.tensor.matmul(opsT[:], v_sb[rb][:, h * VW:(h + 1) * VW],
                                     eT[:], start=(rb == 0), stop=(rb == NCH - 1),
                                     skip_group_check=True)
                # normalize: oT = opsT[0:64] * (1 / opsT[64]), with the
                # reciprocal row broadcast across partitions via a K=1 matmul
                rl = small.tile([1, 512], f32, tag="rl")
                nc.vector.reciprocal(rl[:], opsT[64:65, :])
                rlb = psum.tile([64, 512], f32, tag="rlb", bufs=1)
                nc.tensor.matmul(rlb[:], ones_row[:], rl[:], start=True, stop=True)
                rlb_sb = small.tile([64, 512], f32, tag="rlb_sb")
                nc.scalar.activation(rlb_sb[:], rlb[:], IDENT, bias=0.0, scale=1.0)
                nc.vector.tensor_mul(oT_sb[p][s0:s0 + 64, :], opsT[0:64, :],
                                     rlb_sb[:])

        # ---- stage C: out = oT.T @ Wo ----
        wots = []
        for c in range(NCH):
            t = wstream.tile([P, HIDDEN], bf16, tag=f"wo{c}", bufs=1)
            nc.sync.dma_start(t[:], wo_e[c * P:(c + 1) * P, :])
            wots.append(t)
        for qt in range(NQT):
            for dhalf in range(2):
                ps = psum.tile([P, 512], f32, tag="mm512", bufs=2)
                for c in range(NCH):
                    nc.tensor.matmul(ps[:], oT_sb[c][:, qt * P:(qt + 1) * P],
                                     wots[c][:, dhalf * 512:(dhalf + 1) * 512],
                                     start=(c == 0), stop=(c == NCH - 1))
                ot = work.tile([P, 512], f32, tag="ot")
                nc.vector.tensor_copy(ot[:], ps[:])
                nc.sync.dma_start(
                    out_e[qt * P:(qt + 1) * P, dhalf * 512:(dhalf + 1) * 512], ot[:])


def _get_nc(n_iter=1):
    key = f"nc{n_iter}"
    if key not in _CACHE:
        _CACHE[key] = _build_nc(n_iter)
    return _CACHE[key]


def prepare_in_maps(query_seqs, memory_seqs, positional_encoding, token_mask,
                    content_bias, position_bias, Wq, Wkc, Wkp, Wv, Wo):
    qs = np.asarray(query_seqs, np.float32)
    ms = np.asarray(memory_seqs, np.float32)
    pe = np.asarray(positional_encoding, np.float32)
    tm = np.asarray(token_mask, np.float32)
    scale = np.float32(1.0 / np.sqrt(SPH))

    ref = np.concatenate([ms, qs], axis=1)                      # [B, R, D]
    refT = np.ascontiguousarray(ref.transpose(0, 2, 1))          # [B, D, R]
    queryT = np.ascontiguousarray(qs.transpose(0, 2, 1))         # [B, D, Q]
    posT = np.ascontiguousarray(pe.T)                            # [D, R]
    posT_bf = posT.astype(ml_dtypes.bfloat16)

    bf = ml_dtypes.bfloat16
    wq = np.ascontiguousarray(np.asarray(Wq, np.float32).reshape(HIDDEN, HIDDEN) * scale).astype(bf)
    wkc = np.ascontiguousarray(np.asarray(Wkc, np.float32).reshape(HIDDEN, HIDDEN)).astype(bf)
    wkp = np.ascontiguousarray(np.asarray(Wkp, np.float32).reshape(HIDDEN, HIDDEN)).astype(bf)
    wv = np.ascontiguousarray(np.asarray(Wv, np.float32).reshape(HIDDEN, HIDDEN)).astype(bf)
    wo = np.ascontiguousarray(np.asarray(Wo, np.float32).reshape(HIDDEN, HIDDEN)).astype(bf)

    cbs = (np.asarray(content_bias, np.float32) * scale).reshape(HIDDEN)
    pbs = (np.asarray(position_bias, np.float32) * scale).reshape(HIDDEN)
    cbp = np.ascontiguousarray(cbs.reshape(NPAIR, P).T)          # [128, 8]
    pbp = np.ascontiguousarray(pbs.reshape(NPAIR, P).T)

    # inverse-shifted mask: writing M' into the padded buffer makes the shifted
    # read come out as positions + mask_bias
    mb = (tm[0, 0] * np.float32(NEG_INF)).astype(np.float32)     # [Q, R]
    mp_flat = np.zeros(Q * (R + 1), np.float32)
    mp_flat[Q:] = mb.ravel()
    mp = mp_flat.reshape(Q, R + 1)
    mshift = mp[:, 1:].astype(ml_dtypes.bfloat16)
    mcol = np.ascontiguousarray(mp[:, 0:1]).astype(ml_dtypes.bfloat16)

    in_maps = []
    for b in range(B):
        in_maps.append({
            "refT": np.ascontiguousarray(refT[b]).astype(ml_dtypes.bfloat16),
            "queryT": np.ascontiguousarray(queryT[b]).astype(ml_dtypes.bfloat16),
            "posT": posT_bf,
            "wq": wq, "wkc": wkc, "wkp": wkp, "wv": wv, "wo": wo,
            "cbp": cbp, "pbp": pbp,
            "mshift": mshift, "mcol": mcol,
        })
    return in_maps


def kernel(query_seqs, memory_seqs, positional_encoding, token_mask,
           content_bias, position_bias, Wq, Wkc, Wkp, Wv, Wo):
    from concourse.bass_utils import run_bass_kernel_spmd
    in_maps = prepare_in_maps(query_seqs, memory_seqs, positional_encoding,
                              token_mask, content_bias, position_bias,
                              Wq, Wkc, Wkp, Wv, Wo)
    nc = _get_nc()
    res = run_bass_kernel_spmd(nc, in_maps, core_ids=list(range(B)))
    out = np.stack([np.asarray(res.results[i]["out"], np.float32)
                    for i in range(B)], axis=0)
    return out


# revision 41
# speedup vs baseline: 2.1374x; 2.1374x over previous
"""TransformerXL relative attention on 8 TRN2 NeuronCores, data-parallel over batch.

Problem shapes (hardcoded): B=8, Q=512, M=512, R=1024, HIDDEN=1024, HEADS=16, SPH=64.
Each core computes one batch element end to end; no collectives.

Layout strategy: host passes transposed activations (refT/queryT/posT, [D, *]) so
every matmul has its contraction dim on partitions. rel_shift is exact via a padded
DRAM buffer: writing positions rows into [Q, R+1] (pad col 0) makes the shifted
tensor a contiguous read at element offset Q. The token mask is folded into the
padded buffer on the host (inverse-shifted), so masking costs nothing on device.
Softmax runs without max-subtraction (scores are O(+-30), exp is safe in f32).
"""
import numpy as np
import ml_dtypes

HIDDEN = 1024
HEADS = 16
SPH = 64
B, Q, M = 8, 512, 512
R = Q + M
NEG_INF = -1e9
P = 128
NPAIR = 8   # head pairs
NQT = Q // P
NCH = HIDDEN // P

_CACHE = {}


def _build_nc(n_iter=1):
    import concourse.bass as bass  # noqa: F401
    from concourse import bacc
    import concourse.tile as tile
    import concourse.mybir as mybir

    f32 = mybir.dt.float32
    f32r = mybir.dt.float32r
    bf16 = mybir.dt.bfloat16
    EXP = mybir.ActivationFunctionType.Exp
    IDENT = mybir.ActivationFunctionType.Identity

    nc = bacc.Bacc("TRN2", target_bir_lowering=False, debug=False)

    refT_e = nc.declare_dram_parameter("refT", [HIDDEN, R], bf16, isOutput=False)
    queryT_e = nc.declare_dram_parameter("queryT", [HIDDEN, Q], bf16, isOutput=False)
    posT_e = nc.declare_dram_parameter("posT", [HIDDEN, R], bf16, isOutput=False)
    wq_e = nc.declare_dram_parameter("wq", [HIDDEN, HIDDEN], bf16, isOutput=False)
    wkc_e = nc.declare_dram_parameter("wkc", [HIDDEN, HIDDEN], bf16, isOutput=False)
    wkp_e = nc.declare_dram_parameter("wkp", [HIDDEN, HIDDEN], bf16, isOutput=False)
    wv_e = nc.declare_dram_parameter("wv", [HIDDEN, HIDDEN], bf16, isOutput=False)
    wo_e = nc.declare_dram_parameter("wo", [HIDDEN, HIDDEN], bf16, isOutput=False)
    cbp_e = nc.declare_dram_parameter("cbp", [P, NPAIR], f32, isOutput=False)
    pbp_e = nc.declare_dram_parameter("pbp", [P, NPAIR], f32, isOutput=False)
    mshift_e = nc.declare_dram_parameter("mshift", [Q, R], bf16, isOutput=False)
    mcol_e = nc.declare_dram_parameter("mcol", [Q, 1], bf16, isOutput=False)
    out_e = nc.declare_dram_parameter("out", [Q, HIDDEN], f32, isOutput=True)

    with tile.TileContext(nc) as tc:
        from contextlib import ExitStack
        ctx = ExitStack()
        dram = ctx.enter_context(tc.tile_pool(name="dram", bufs=1, space="DRAM"))
        # per-head padded DRAM buffers for the rel_shift round trip (tile pool so
        # Tile tracks write->read deps, incl. the tile-crossing wrap reads)
        pads = [dram.tile([Q * (R + 1)], bf16, tag=f"pad{h}", name=f"pad{h}")
                for h in range(HEADS)]
        pad_rows = [t[:].rearrange("(q c) -> q c", c=R + 1) for t in pads]
        shift_views = [t[Q:Q + Q * R].rearrange("(q c) -> q c", c=R) for t in pads]
        const = ctx.enter_context(tc.tile_pool(name="const", bufs=1))
        resid = ctx.enter_context(tc.tile_pool(name="resid", bufs=1))
        wstream = ctx.enter_context(tc.tile_pool(name="wstream", bufs=2))
        psum = ctx.enter_context(tc.tile_pool(name="psum", bufs=1, space="PSUM"))
        work = ctx.enter_context(tc.tile_pool(name="work", bufs=2))
        small = ctx.enter_context(tc.tile_pool(name="small", bufs=3))

        # ---- resident loads (refT first: V depends on it) ----
        refT_sb = []
        for c in range(NCH):
            t = resid.tile([P, R], bf16, tag=f"refT{c}")
            nc.sync.dma_start(t[:], refT_e[c * P:(c + 1) * P, :])
            refT_sb.append(t)
        import numpy as _np
        import ml_dtypes as _mld
        ident_d = nc.inline_tensor(_np.eye(P, dtype=_mld.bfloat16), name="ident_d")
        ident = const.tile([P, P], bf16, tag="ident", name="ident")
        nc.sync.dma_start(ident[:], ident_d[:, :])

        state = {}
        for _it in range(n_iter):
            _build_body(nc, tc, mybir, ctx, const, resid, wstream, psum, work,
                        small, dram, pads, pad_rows, shift_views, state,
                        refT_sb, (cbp_e, pbp_e, mshift_e, posT_e, queryT_e,
                                  mcol_e),
                        wq_e, wkc_e, wkp_e, wv_e, wo_e, out_e, ident)
        ctx.close()

    nc.compile()
    return nc


def _build_body(nc, tc, mybir, ctx, const, resid, wstream, psum, work, small,
                dram, pads, pad_rows, shift_views, state, refT_sb, deferred,
                wq_e, wkc_e, wkp_e, wv_e, wo_e, out_e, ident):
        f32 = mybir.dt.float32
        bf16 = mybir.dt.bfloat16
        EXP = mybir.ActivationFunctionType.Exp
        IDENT = mybir.ActivationFunctionType.Identity
        VW = 65  # 64 v columns + 1 ones column per head (softmax denominator)

        # ---- stage V: v_sb[rt][:, 65h:65h+64] = (ref @ Wv)[rt], col 65h+64 = 1
        v_sb = []
        for rt in range(NCH):
            t = resid.tile([P, HEADS * VW], bf16, tag=f"v{rt}", name=f"v{rt}")
            nc.vector.memset(t[:].rearrange("p (h w) -> p h w", w=VW)[:, :, 64:65],
                             1.0)
            v_sb.append(t)
        wvts = []
        for c in range(NCH):
            t = wstream.tile([P, HIDDEN], bf16, tag=f"wvh{c}", bufs=1)
            nc.sync.dma_start(t[:], wv_e[c * P:(c + 1) * P, :])
            wvts.append(t)
        if not state:
            cbp_e, pbp_e, mshift_e, posT_e, queryT_e, mcol_e = deferred
            cbp = const.tile([P, NPAIR], f32, tag="cbp", name="cbp")
            nc.sync.dma_start(cbp[:], cbp_e[:, :])
            pbp = const.tile([P, NPAIR], f32, tag="pbp", name="pbp")
            nc.sync.dma_start(pbp[:], pbp_e[:, :])
            mshift_sb = []
            for qt in range(NQT):
                mt = const.tile([P, R], bf16, tag=f"mshift{qt}", name=f"ms{qt}")
                nc.sync.dma_start(mt[:], mshift_e[qt * P:(qt + 1) * P, :])
                mshift_sb.append(mt)
            posT_sb = []
            for c in range(NCH):
                t = resid.tile([P, R], bf16, tag=f"posT{c}", name=f"pT{c}")
                nc.sync.dma_start(t[:], posT_e[c * P:(c + 1) * P, :])
                posT_sb.append(t)
            queryT_sb = []
            for c in range(NCH):
                t = resid.tile([P, Q], bf16, tag=f"queryT{c}", name=f"qT{c}")
                nc.sync.dma_start(t[:], queryT_e[c * P:(c + 1) * P, :])
                queryT_sb.append(t)
            with nc.allow_non_contiguous_dma(reason="one-time pad columns"):
                for hh in range(HEADS):
                    nc.gpsimd.dma_start(pad_rows[hh][:, 0:1], mcol_e[:, :])
            state.update(cbp=cbp, pbp=pbp, mshift_sb=mshift_sb,
                         posT_sb=posT_sb, queryT_sb=queryT_sb)
        cbp = state["cbp"]; pbp = state["pbp"]
        mshift_sb = state["mshift_sb"]
        posT_sb = state["posT_sb"]; queryT_sb = state["queryT_sb"]

        for rt in range(NCH):
            for half in range(2):
                vps = psum.tile([P, 512], f32, tag="mm512", bufs=2)
                for c in range(NCH):
                    nc.tensor.matmul(vps[:], refT_sb[c][:, rt * P:(rt + 1) * P],
                                     wvts[c][:, half * 512:(half + 1) * 512],
                                     start=(c == 0), stop=(c == NCH - 1))
                # strided copy into the 65-wide head slots
                dst = v_sb[rt][:, half * 8 * VW:(half * 8 + 8) * VW]
                dst = dst.rearrange("p (h w) -> p h w", w=VW)[:, :, 0:64]
                nc.scalar.activation(dst, vps[:].rearrange("p (h w) -> p h w", w=64),
                                     IDENT, bias=0.0, scale=1.0)


        # ---- per head-pair ----
        oT_sb = []
        for p in range(NPAIR):
            oT_sb.append(resid.tile([P, Q], bf16, tag=f"oT{p}", name=f"oT{p}"))

        for p in range(NPAIR):
            hs0 = p * P
            # pair-column weight loads: one DMA each, [128, 8*128] with chunk c
            # at columns [c*128, (c+1)*128)
            def _pair_w(w_e, name):
                # host pre-permuted: rows [p*128,(p+1)*128) hold this pair's
                # column block chunk-major, so the load is fully contiguous
                t = wstream.tile([P, HIDDEN], bf16, tag=f"wp_{name}", bufs=3,
                                 name=f"wp_{name}")
                nc.sync.dma_start(t[:], w_e[hs0:hs0 + P, :])
                return t
            wkct = _pair_w(wkc_e, "kc")
            wkpt = _pair_w(wkp_e, "kp")
            wqt = _pair_w(wq_e, "q")

            kc_sb = work.tile([P, R], bf16, tag="kc_sb", bufs=3)
            kp_sb = work.tile([P, R], bf16, tag="kp_sb", bufs=3)
            for rhalf in range(2):
                ps = psum.tile([P, 512], f32, tag="mm512", bufs=2)
                for c in range(NCH):
                    nc.tensor.matmul(ps[:], wkct[:, c * P:(c + 1) * P],
                                     refT_sb[c][:, rhalf * 512:(rhalf + 1) * 512],
                                     start=(c == 0), stop=(c == NCH - 1))
                nc.vector.tensor_copy(kc_sb[:, rhalf * 512:(rhalf + 1) * 512], ps[:])
            for rhalf in range(2):
                ps = psum.tile([P, 512], f32, tag="mm512", bufs=2)
                for c in range(NCH):
                    nc.tensor.matmul(ps[:], wkpt[:, c * P:(c + 1) * P],
                                     posT_sb[c][:, rhalf * 512:(rhalf + 1) * 512],
                                     start=(c == 0), stop=(c == NCH - 1))
                nc.vector.tensor_copy(kp_sb[:, rhalf * 512:(rhalf + 1) * 512], ps[:])
            qc_sb = work.tile([P, Q], bf16, tag="qc_sb", bufs=3)
            qp_sb = work.tile([P, Q], bf16, tag="qp_sb", bufs=3)
            ps = psum.tile([P, 512], f32, tag="mm512", bufs=2)
            for c in range(NCH):
                nc.tensor.matmul(ps[:], wqt[:, c * P:(c + 1) * P], queryT_sb[c][:],
                                 start=(c == 0), stop=(c == NCH - 1))
            nc.vector.tensor_scalar_add(qc_sb[:], ps[:], cbp[:, p:p + 1])
            nc.vector.tensor_scalar_add(qp_sb[:], ps[:], pbp[:, p:p + 1])

            # both heads of the pair interleaved so the K=64 matmuls of
            # u=0 (array rows 0-63) and u=1 (rows 64-127) sit adjacent in the
            # PE queue and row-pack
            h0, h1 = 2 * p, 2 * p + 1
            # pass 1: position scores -> padded DRAM rows (bf16), one head at
            # a time so each head's transposed read-back issues as early as
            # possible (h1's pass 1 hides h0's pad->transpose latency)
            shAs = [None, None]
            for u in range(2):
                for qt in range(NQT):
                    padt = work.tile([P, R], bf16, tag="padt", bufs=6,
                                     name=f"padt{u}")
                    for rhalf in range(2):
                        pps = psum.tile([P, 512], f32, tag="pps", bufs=2)
                        nc.tensor.matmul(
                            pps[:],
                            qp_sb[u * 64:u * 64 + 64, qt * P:(qt + 1) * P],
                            kp_sb[u * 64:u * 64 + 64,
                                  rhalf * 512:(rhalf + 1) * 512],
                            start=True, stop=True)
                        nc.vector.tensor_add(
                            padt[:, rhalf * 512:(rhalf + 1) * 512], pps[:],
                            mshift_sb[qt][:, rhalf * 512:(rhalf + 1) * 512])
                    nc.scalar.dma_start(
                        pad_rows[2 * p + u][qt * P:(qt + 1) * P, 1:], padt[:])
                shA = work.tile([P, NCH * 512], bf16, tag="shA", bufs=3,
                                name=f"shA{u}")
                nc.scalar.dma_start(
                    shA[:].rearrange("p (b q) -> p b q", q=512),
                    shift_views[2 * p + u][:, :], transpose=True)
                shAs[u] = shA
            opsTs = [psum.tile([VW, 512], f32, tag="opsT", bufs=2,
                               name=f"opsT{u}") for u in range(2)]
            for rb in range(NCH):
                for u in range(2):
                    cpsT = psum.tile([P, 512], f32, tag="cpsT", bufs=2)
                    nc.tensor.matmul(cpsT[:],
                                     kc_sb[u * 64:u * 64 + 64, rb * P:(rb + 1) * P],
                                     qc_sb[u * 64:u * 64 + 64, :],
                                     start=True, stop=False,
                                     skip_group_check=True)
                    nc.tensor.matmul(cpsT[:], ident[:],
                                     shAs[u][:, rb * 512:(rb + 1) * 512],
                                     start=False, stop=True, skip_group_check=True)
                    eT = work.tile([P, 512], bf16, tag="eT", bufs=4)
                    nc.scalar.activation(eT[:], cpsT[:], EXP, bias=0.0, scale=1.0)
                    nc.tensor.matmul(opsTs[u][0:VW, :],
                                     v_sb[rb][:, (2 * p + u) * VW:(2 * p + u + 1) * VW],
                                     eT[:], start=(rb == 0), stop=(rb == NCH - 1),
                                     skip_group_check=True)
            for u in range(2):
                # normalize: oT = opsT[0:64] * (1 / opsT[64]); broadcast the
                # reciprocal row across partitions on the idle gpsimd engine
                rl = small.tile([1, 512], f32, tag="rl")
                nc.vector.reciprocal(rl[:], opsTs[u][64:65, :])
                rlb_sb = small.tile([64, 512], f32, tag="rlb_sb")
                nc.gpsimd.partition_broadcast(rlb_sb[:], rl[:])
                nc.vector.tensor_mul(oT_sb[p][u * 64:u * 64 + 64, :],
                                     opsTs[u][0:64, :], rlb_sb[:])

        # ---- stage C: out = oT.T @ Wo ----
        wots = []
        for c in range(NCH):
            t = wstream.tile([P, HIDDEN], bf16, tag=f"wo{c}", bufs=1)
            nc.sync.dma_start(t[:], wo_e[c * P:(c + 1) * P, :])
            wots.append(t)
        for qt in range(NQT):
            for dhalf in range(2):
                ps = psum.tile([P, 512], f32, tag="mm512", bufs=2)
                for c in range(NCH):
                    nc.tensor.matmul(ps[:], oT_sb[c][:, qt * P:(qt + 1) * P],
                                     wots[c][:, dhalf * 512:(dhalf + 1) * 512],
                                     start=(c == 0), stop=(c == NCH - 1))
                ot = work.tile([P, 512], f32, tag="ot")
                nc.scalar.activation(ot[:], ps[:], IDENT, bias=0.0, scale=1.0)
                nc.sync.dma_start(
                    out_e[qt * P:(qt + 1) * P, dhalf * 512:(dhalf + 1) * 512], ot[:])


def _get_nc(n_iter=1):
    key = f"nc{n_iter}"
    if key not in _CACHE:
        _CACHE[key] = _build_nc(n_iter)
    return _CACHE[key]


def prepare_in_maps(query_seqs, memory_seqs, positional_encoding, token_mask,
                    content_bias, position_bias, Wq, Wkc, Wkp, Wv, Wo):
    qs = np.asarray(query_seqs, np.float32)
    ms = np.asarray(memory_seqs, np.float32)
    pe = np.asarray(positional_encoding, np.float32)
    tm = np.asarray(token_mask, np.float32)
    scale = np.float32(1.0 / np.sqrt(SPH))

    ref = np.concatenate([ms, qs], axis=1)                      # [B, R, D]
    refT = np.ascontiguousarray(ref.transpose(0, 2, 1))          # [B, D, R]
    queryT = np.ascontiguousarray(qs.transpose(0, 2, 1))         # [B, D, Q]
    posT = np.ascontiguousarray(pe.T)                            # [D, R]
    posT_bf = posT.astype(ml_dtypes.bfloat16)

    bf = ml_dtypes.bfloat16

    def _pair_permute(w):
        # [D, H*S] -> rows p*128..(p+1)*128 = pair p's 128 columns, chunk-major:
        # w_pre[row, c*128+col] = w[c*128+row, p*128+col]
        return np.ascontiguousarray(
            w.reshape(NCH, P, NPAIR, P).transpose(2, 1, 0, 3).reshape(
                HIDDEN, HIDDEN))

    wq = _pair_permute(np.asarray(Wq, np.float32).reshape(HIDDEN, HIDDEN) * scale).astype(bf)
    wkc = _pair_permute(np.asarray(Wkc, np.float32).reshape(HIDDEN, HIDDEN)).astype(bf)
    wkp = _pair_permute(np.asarray(Wkp, np.float32).reshape(HIDDEN, HIDDEN)).astype(bf)
    wv = np.ascontiguousarray(np.asarray(Wv, np.float32).reshape(HIDDEN, HIDDEN)).astype(bf)
    wo = np.ascontiguousarray(np.asarray(Wo, np.float32).reshape(HIDDEN, HIDDEN)).astype(bf)

    cbs = (np.asarray(content_bias, np.float32) * scale).reshape(HIDDEN)
    pbs = (np.asarray(position_bias, np.float32) * scale).reshape(HIDDEN)
    cbp = np.ascontiguousarray(cbs.reshape(NPAIR, P).T)          # [128, 8]
    pbp = np.ascontiguousarray(pbs.reshape(NPAIR, P).T)

    # inverse-shifted mask: writing M' into the padded buffer makes the shifted
    # read come out as positions + mask_bias
    mb = (tm[0, 0] * np.float32(NEG_INF)).astype(np.float32)     # [Q, R]
    mp_flat = np.zeros(Q * (R + 1), np.float32)
    mp_flat[Q:] = mb.ravel()
    mp = mp_flat.reshape(Q, R + 1)
    mshift = mp[:, 1:].astype(ml_dtypes.bfloat16)
    mcol = np.ascontiguousarray(mp[:, 0:1]).astype(ml_dtypes.bfloat16)

    in_maps = []
    for b in range(B):
        in_maps.append({
            "refT": np.ascontiguousarray(refT[b]).astype(ml_dtypes.bfloat16),
            "queryT": np.ascontiguousarray(queryT[b]).astype(ml_dtypes.bfloat16),
            "posT": posT_bf,
            "wq": wq, "wkc": wkc, "wkp": wkp, "wv": wv, "wo": wo,
            "cbp": cbp, "pbp": pbp,
            "mshift": mshift, "mcol": mcol,
        })
    return in_maps


def kernel(query_seqs, memory_seqs, positional_encoding, token_mask,
           content_bias, position_bias, Wq, Wkc, Wkp, Wv, Wo):
    from concourse.bass_utils import run_bass_kernel_spmd
    in_maps = prepare_in_maps(query_seqs, memory_seqs, positional_encoding,
                              token_mask, content_bias, position_bias,
                              Wq, Wkc, Wkp, Wv, Wo)
    nc = _get_nc()
    res = run_bass_kernel_spmd(nc, in_maps, core_ids=list(range(B)))
    out = np.stack([np.asarray(res.results[i]["out"], np.float32)
                    for i in range(B)], axis=0)
    return out
